# revision 32
# baseline (speedup 1.0000x reference)
"""Ernie4.5-VL MoE layer on 8 Trainium2 NeuronCores (Bass/Tile).

Sharding (expert-parallel, per sharding_hint):
  - 16 stacked experts (2 modalities x 8) -> 2 per core. Core c handles
    stacked experts {2c, 2c+1}; both always belong to modality m = c//4.
  - Host permutes that modality's gate columns / bias so the core's two
    experts sit at local positions 0,1. Softmax/top-k are permutation
    equivariant, so on-device routing over the permuted 8 columns is exact.
  - Shared-expert FFN is tensor-parallel along the intermediate dim
    (2048/8 = 256 columns per core).
  - Every core emits a partial [512, 2048] output; the host sums the 8
    partials (the unshard step for this sharding).

On-device per core:
  - routing: scores = softmax(x @ gate) in fp32, top-2 of 8 via two maxes
    on (scores + bias), renormalized on the original scores, masked by
    modality -> per-token combine weights g0, g1 for the local experts.
  - hT_e = silu(Wg_e^T x^T) * (Wu_e^T x^T) for both experts (fp32r
    matmuls, fp32 PSUM accumulate) stored transposed [I, tokens].
  - y = g0*(hT_0^T @ Wd_0) + g1*(hT_1^T @ Wd_1) + shared partial, with all
    down-projections accumulated in PSUM per (token-tile, h-chunk).

fp32r ("relaxed" fp32) runs the PE at full (bf16) rate with ~1.5e-4
matmul relative error (hardware-probed); routing stays in full fp32 so
top-k selection is bit-stable against the jax reference.
"""

import sys

sys.path.insert(0, "/opt/trn_rl_repo")

import numpy as np

import concourse.bass as bass  # noqa: F401
import concourse.tile as tile
from concourse import bacc, mybir
from concourse import bass_utils
from concourse.bass import ts, ds

P = 128  # partitions
NTOK = 512  # tokens
NTT = NTOK // P  # token tiles
H = 2048  # hidden
KC = H // P  # contraction chunks over H
I_FF = 1024  # expert ffn intermediate
NIC = I_FF // P  # intermediate chunks (experts)
IS = 2048  # shared ffn intermediate (total)
NCORES = 8
IS_SL = IS // NCORES  # shared intermediate slice per core
NIC_S = IS_SL // P
HCW = 512  # output h-chunk width
NHC = H // HCW
E = 8  # experts per modality

f32 = mybir.dt.float32
f32r = mybir.dt.float32r
AF = mybir.ActivationFunctionType
ALU = mybir.AluOpType


def _build_nc():
    nc = bacc.Bacc(
        "TRN2",
        target_bir_lowering=False,
        debug=False,
        enable_asserts=False,
        num_devices=NCORES,
    )
    xTr = nc.dram_tensor("xTr", [H, NTOK], f32r, kind="ExternalInput").ap()
    xTf = nc.dram_tensor("xTf", [H, NTOK], f32, kind="ExternalInput").ap()
    gate = nc.dram_tensor("gate", [H, E], f32, kind="ExternalInput").ap()
    bias_rep = nc.dram_tensor("bias_rep", [P, E], f32, kind="ExternalInput").ap()
    mask_pc = nc.dram_tensor("mask_pc", [P, NTT], f32, kind="ExternalInput").ap()
    wg = nc.dram_tensor("wg", [2, H, I_FF], f32r, kind="ExternalInput").ap()
    wu = nc.dram_tensor("wu", [2, H, I_FF], f32r, kind="ExternalInput").ap()
    wd = nc.dram_tensor("wd", [2, I_FF, H], f32r, kind="ExternalInput").ap()
    wsg = nc.dram_tensor("wsg", [H, IS_SL], f32r, kind="ExternalInput").ap()
    wsu = nc.dram_tensor("wsu", [H, IS_SL], f32r, kind="ExternalInput").ap()
    wsd = nc.dram_tensor("wsd", [IS_SL, H], f32r, kind="ExternalInput").ap()
    y = nc.dram_tensor("y", [NTOK, H], f32, kind="ExternalOutput").ap()

    xTr_v = xTr.rearrange("(o p) t -> p o t", p=P)  # [128, 16, 512]
    xTf_v = xTf.rearrange("(o p) t -> p o t", p=P)
    gate_v = gate.rearrange("(o p) e -> p o e", p=P)  # [128, 16, 8]
    wg_v = wg.rearrange("e (o p) i -> p e o i", p=P)  # [128, 2, 16, 1024]
    wu_v = wu.rearrange("e (o p) i -> p e o i", p=P)
    wd_v = wd.rearrange("e (o p) h -> p e o h", p=P)  # [128, 2, 8, 2048]
    wsg_v = wsg.rearrange("(o p) i -> p o i", p=P)  # [128, 16, 256]
    wsu_v = wsu.rearrange("(o p) i -> p o i", p=P)
    wsd_v = wsd.rearrange("(o p) h -> p o h", p=P)  # [128, 2, 2048]
    y_v = y.rearrange("(tt p) h -> p tt h", p=P)  # [128, 4, 2048]

    with tile.TileContext(nc) as tc:
        with (
            tc.tile_pool(name="const", bufs=1) as cp,
            tc.tile_pool(name="rtp", bufs=2) as rtp,
            tc.tile_pool(name="wgwu", bufs=2) as wp,
            tc.tile_pool(name="silp", bufs=2) as silp,
            tc.tile_pool(name="outp", bufs=4) as outp,
        ):
            # Pool release must be LIFO; allocate in reverse lifetime order:
            # wdp (lives to kernel end) before psA (to shared-ffn end) before
            # psr/xfp (die after routing finalize).
            wdp = tc.alloc_tile_pool(name="wdp", bufs=4)
            # ---------- persistent SBUF ----------
            # xTr on the POOL/SWDGE path: keeps the shared HWDGE free for the
            # routing xf stream + first wg tiles at kernel start.
            xTr_sb = cp.tile([P, KC, NTOK], f32r)
            for j in range(8):  # split 4MB load across DMA queues
                nc.gpsimd.dma_start(
                    xTr_sb[:, ts(j, KC // 8), :], xTr_v[:, ts(j, KC // 8), :]
                )
            gate_sb = cp.tile([P, KC, E], f32)
            nc.sync.dma_start(gate_sb[:], gate_v[:])
            bias_sb = cp.tile([P, E], f32)
            nc.sync.dma_start(bias_sb[:], bias_rep[:])
            mask_sb = cp.tile([P, NTT], f32)
            nc.sync.dma_start(mask_sb[:], mask_pc[:])
            hT0 = cp.tile([P, NIC, NTOK], f32r)
            hT1 = cp.tile([P, NIC, NTOK], f32r)
            hsT = cp.tile([P, NIC_S, NTOK], f32r)
            cw_sb = cp.tile([P, NTT, 2], f32)

            # ---------- routing (fp32), interleaved with phase A ----------
            # The PE consumes its stream in order, so the routing matmuls are
            # split into two waves woven between the expert FFN phases; their
            # xf feed is always DMA-resident by the time the PE reaches them.
            def xf_load(kc, eng=None):
                xf = xfp.tile([P, NTOK], f32, tag="xf", bufs=3, name=f"xf{kc}")
                (eng or nc.sync).dma_start(xf[:], xTf_v[:, kc, :])
                return xf

            def routing_wave(ps_s, xf_tiles, kc_lo, kc_hi):
                for kc in range(kc_lo, kc_hi):
                    xf = xf_tiles[kc]
                    for tt in range(NTT):
                        nc.tensor.matmul(
                            ps_s[tt][:],
                            xf[:, ts(tt, P)],
                            gate_sb[:, kc, :],
                            start=(kc == 0),
                            stop=(kc == KC - 1),
                        )

            def routing_finalize(ps_s):
                for tt in range(NTT):
                    s = ps_s[tt]
                    nmx = rtp.tile([P, 1], f32)
                    nc.vector.tensor_reduce(
                        nmx[:], s[:], mybir.AxisListType.X, ALU.max, negate=True
                    )
                    ex = rtp.tile([P, E], f32)
                    nc.scalar.activation(ex[:], s[:], AF.Exp, bias=nmx[:])
                    ssum = rtp.tile([P, 1], f32)
                    nc.vector.tensor_reduce(
                        ssum[:], ex[:], mybir.AxisListType.X, ALU.add
                    )
                    rs = rtp.tile([P, 1], f32)
                    nc.vector.reciprocal(rs[:], ssum[:])
                    pr = rtp.tile([P, E], f32)
                    nc.vector.tensor_scalar_mul(pr[:], ex[:], rs[:])
                    bb = rtp.tile([P, E], f32)
                    nc.vector.tensor_add(bb[:], pr[:], bias_sb[:])
                    m1 = rtp.tile([P, 1], f32)
                    nc.vector.tensor_reduce(
                        m1[:], bb[:], mybir.AxisListType.X, ALU.max
                    )
                    k1 = rtp.tile([P, E], f32)
                    nc.vector.tensor_scalar(k1[:], bb[:], m1[:], None, ALU.is_equal)
                    b2 = rtp.tile([P, E], f32)
                    nc.vector.scalar_tensor_tensor(
                        b2[:], k1[:], -1.0e9, bb[:], ALU.mult, ALU.add
                    )
                    m2 = rtp.tile([P, 1], f32)
                    nc.vector.tensor_reduce(
                        m2[:], b2[:], mybir.AxisListType.X, ALU.max
                    )
                    k2 = rtp.tile([P, E], f32)
                    nc.vector.tensor_scalar(k2[:], b2[:], m2[:], None, ALU.is_equal)
                    sel = rtp.tile([P, E], f32)
                    nc.vector.tensor_add(sel[:], k1[:], k2[:])
                    w = rtp.tile([P, E], f32)
                    nc.vector.tensor_mul(w[:], pr[:], sel[:])
                    ws = rtp.tile([P, 1], f32)
                    nc.vector.tensor_reduce(
                        ws[:], w[:], mybir.AxisListType.X, ALU.add
                    )
                    rw = rtp.tile([P, 1], f32)
                    nc.vector.reciprocal(rw[:], ws[:])
                    sc = rtp.tile([P, 1], f32)
                    nc.vector.tensor_mul(sc[:], rw[:], mask_sb[:, tt : tt + 1])
                    nc.vector.tensor_scalar(
                        cw_sb[:, tt, :], w[:, 0:2], sc[:], None, ALU.mult
                    )

            # ---------- phase A + routing waves ----------
            # psr (4 banks) + psA (2x2 banks) coexist: exactly 8 PSUM banks.
            psA = tc.alloc_tile_pool(name="psA", bufs=2, space="PSUM")
            psr = tc.alloc_tile_pool(name="psr", bufs=1, space="PSUM")
            xfp = tc.alloc_tile_pool(name="xfp", bufs=4)
            ps_s = [psr.tile([P, E], f32, name=f"ps_s{tt}") for tt in range(NTT)]
            xf_tiles = {kc: xf_load(kc) for kc in range(3)}  # kc 0..2 on sync

            def ffn_load(src_g, src_u, ic):
                wg_t = wp.tile([P, KC, P], f32r, tag="wgt", name="wg_t")
                wu_t = wp.tile([P, KC, P], f32r, tag="wut", name="wu_t")
                for j in range(4):  # split 1MB loads across queues; wg on
                    # sync HWDGE, wu on ACT HWDGE.
                    nc.sync.dma_start(
                        wg_t[:, ts(j, KC // 4), :],
                        src_g[:, ts(j, KC // 4), ts(ic, P)],
                    )
                    nc.scalar.dma_start(
                        wu_t[:, ts(j, KC // 4), :],
                        src_u[:, ts(j, KC // 4), ts(ic, P)],
                    )
                return wg_t, wu_t

            def ffn_up(dst, n_ic, src_g, src_u, post_ic=None, tiles0=None):
                """dst[:, ic, :] = silu(g) * u, transposed [I-chunk, tokens].

                DMA issue for iteration ic+1 is placed BEFORE iteration ic's
                silu: the silu's sequencer-level wait on PSUM would otherwise
                hold back the next weight loads on the same (ACT) engine.
                """
                silus = []
                tiles = {0: tiles0 if tiles0 is not None else ffn_load(src_g, src_u, 0)}
                for ic in range(n_ic):
                    if ic + 1 < n_ic:
                        tiles[ic + 1] = ffn_load(src_g, src_u, ic + 1)
                    wg_t, wu_t = tiles.pop(ic)
                    ps_g = psA.tile([P, NTOK], f32, tag="psg", name="ps_g")
                    ps_u = psA.tile([P, NTOK], f32, tag="psu", name="ps_u")
                    for kc in range(KC):
                        nc.tensor.matmul(
                            ps_g[:],
                            wg_t[:, kc, :],
                            xTr_sb[:, kc, :],
                            start=(kc == 0),
                            stop=(kc == KC - 1),
                        )
                    for kc in range(KC):
                        nc.tensor.matmul(
                            ps_u[:],
                            wu_t[:, kc, :],
                            xTr_sb[:, kc, :],
                            start=(kc == 0),
                            stop=(kc == KC - 1),
                        )
                    sil = silp.tile([P, NTOK], f32, tag="sil", name="sil")
                    silus.append(nc.scalar.activation(sil[:], ps_g[:], AF.Silu))
                    nc.vector.tensor_mul(dst[:, ic, :], sil[:], ps_u[:])
                    if post_ic is not None:
                        post_ic(ic)
                return silus

            # Routing is woven INTO expert 0: kc 0..1 run up front, then two
            # kc per expert-0 iteration. Each xf chunk lands during the
            # preceding iteration's compute, so the (in-order) PE never waits
            # on the routing feed.
            routing_wave(ps_s, xf_tiles, 0, 2)
            for kc in range(3, KC):
                xf_tiles[kc] = xf_load(kc, eng=nc.gpsimd)

            def expert0_post_ic(ic):
                routing_wave(ps_s, xf_tiles, min(2 * ic + 2, KC), min(2 * ic + 4, KC))

            ffn_up(hT0, NIC, wg_v[:, 0], wu_v[:, 0], post_ic=expert0_post_ic)
            routing_finalize(ps_s)
            # xf + routing psum are dead from here.
            xfp.release()
            psr.release()

            def wd_load(hc, e, eng, dmas=None):
                t = wdp.tile([P, NIC, HCW], f32r, tag="wdt", name=f"wd{e}_{hc}")
                for j in range(4):
                    d = eng.dma_start(
                        t[:, ts(j, NIC // 4), :],
                        wd_v[:, e, ts(j, NIC // 4), ds(hc * HCW, HCW)],
                    )
                    if dmas is not None:
                        dmas.append(d)
                return t

            def wsd_load(hc, eng, dmas=None):
                t = wdp.tile(
                    [P, NIC_S, HCW], f32r, tag="wsdt", bufs=2, name=f"wsd_{hc}"
                )
                d = eng.dma_start(t[:], wsd_v[:, :, ds(hc * HCW, HCW)])
                if dmas is not None:
                    dmas.append(d)
                return t

            ffn_up(hT1, NIC, wg_v[:, 1], wu_v[:, 1])
            sh_silus = ffn_up(hsT, NIC_S, wsg_v, wsu_v)
            # Pre-issue ALL phase-B weights on the otherwise-idle POOL/SWDGE
            # path: its sequencer is not paced by phase-A compute, so these
            # fill the DMA hole at the A->B boundary. The first two hc's
            # bursts are explicitly held back (dep on the shared-FFN silu) so
            # they don't jump the FIFO ahead of late phase-A weight feeds;
            # hc 2..3 are naturally paced by wdt slot reuse.
            from concourse.tile_rust import add_dep_helper

            marker = sh_silus[0].ins
            early: list = []
            wd_pre = {}
            for hc in range(NHC):
                dmas = early if hc < 2 else None
                wd_pre[hc] = (
                    wd_load(hc, 0, nc.gpsimd, dmas),
                    wd_load(hc, 1, nc.gpsimd, dmas),
                    wsd_load(hc, nc.gpsimd, dmas),
                )
            for d in early:
                add_dep_helper(d.ins, marker, reason="pace phase-B wd prefetch")
            psA.release()

            # ---------- phase B: down-proj + combine ----------
            with tc.tile_pool(name="psB", bufs=2, space="PSUM") as psB:
                for hc in range(NHC):
                    wd0, wd1, wsd_t = wd_pre.pop(hc)
                    for tt in range(NTT):
                        ps0 = psB.tile([P, HCW], f32, tag="py0")
                        ps1 = psB.tile([P, HCW], f32, tag="py1")
                        pss = psB.tile([P, HCW], f32, tag="pys")
                        for ic in range(NIC):
                            nc.tensor.matmul(
                                ps0[:],
                                hT0[:, ic, ts(tt, P)],
                                wd0[:, ic, :],
                                start=(ic == 0),
                                stop=(ic == NIC - 1),
                            )
                        for ic in range(NIC):
                            nc.tensor.matmul(
                                ps1[:],
                                hT1[:, ic, ts(tt, P)],
                                wd1[:, ic, :],
                                start=(ic == 0),
                                stop=(ic == NIC - 1),
                            )
                        for ic in range(NIC_S):
                            nc.tensor.matmul(
                                pss[:],
                                hsT[:, ic, ts(tt, P)],
                                wsd_t[:, ic, :],
                                start=(ic == 0),
                                stop=(ic == NIC_S - 1),
                            )
                        # only one PSUM operand allowed per DVE op: scale ps0
                        # on ACT, then fold ps1 and pss in on DVE.
                        t_a = outp.tile([P, HCW], f32, tag="otmp")
                        nc.scalar.activation(
                            t_a[:], ps0[:], AF.Identity, scale=cw_sb[:, tt, 0:1]
                        )
                        t_b = outp.tile([P, HCW], f32, tag="otmp")
                        nc.vector.scalar_tensor_tensor(
                            t_b[:], ps1[:], cw_sb[:, tt, 1:2], t_a[:], ALU.mult, ALU.add
                        )
                        out_t = outp.tile([P, HCW], f32, tag="otmp")
                        nc.vector.tensor_add(out_t[:], t_b[:], pss[:])
                        nc.sync.dma_start(y_v[:, tt, ds(hc * HCW, HCW)], out_t[:])
            wdp.release()

    return nc


_CACHE: dict = {}


def _get_compiled():
    if "nc" not in _CACHE:
        nc = _build_nc()
        nc.compile()
        _CACHE["nc"] = nc
    return _CACHE["nc"]


def _shard_inputs(inputs) -> list[dict]:
    hs = np.asarray(inputs["hidden_states"], np.float32).reshape(-1, H)
    xT = np.ascontiguousarray(hs.T)
    v = np.asarray(inputs["visual_token_mask"]).reshape(-1).astype(bool)
    bias = np.asarray(inputs["bias"], np.float32)
    W_gate = np.asarray(inputs["W_gate"], np.float32)
    W_up = np.asarray(inputs["W_up"], np.float32)
    W_down = np.asarray(inputs["W_down"], np.float32)
    Ws_gate = np.asarray(inputs["Ws_gate"], np.float32)
    Ws_up = np.asarray(inputs["Ws_up"], np.float32)
    Ws_down = np.asarray(inputs["Ws_down"], np.float32)

    in_maps = []
    for c in range(NCORES):
        m = c // 4
        p0 = (2 * c) % 8
        perm = [p0, p0 + 1] + [j for j in range(E) if j not in (p0, p0 + 1)]
        wgate_full = inputs["w_text_gate"] if m == 0 else inputs["w_vis_gate"]
        gate_c = np.ascontiguousarray(np.asarray(wgate_full, np.float32)[:, perm])
        bias_rep = np.tile(bias[m, perm][None, :], (P, 1))
        mask_f = (v if m == 1 else ~v).astype(np.float32)
        mask_pc = np.ascontiguousarray(mask_f.reshape(NTT, P).T)
        sl = slice(c * IS_SL, (c + 1) * IS_SL)
        in_maps.append(
            {
                "xTr": xT,
                "xTf": xT,
                "gate": gate_c,
                "bias_rep": np.ascontiguousarray(bias_rep),
                "mask_pc": mask_pc,
                "wg": np.ascontiguousarray(W_gate[m, [p0, p0 + 1]]),
                "wu": np.ascontiguousarray(W_up[m, [p0, p0 + 1]]),
                "wd": np.ascontiguousarray(W_down[m, [p0, p0 + 1]]),
                "wsg": np.ascontiguousarray(Ws_gate[:, sl]),
                "wsu": np.ascontiguousarray(Ws_up[:, sl]),
                "wsd": np.ascontiguousarray(Ws_down[sl, :]),
            }
        )
    return in_maps


def kernel(**inputs) -> np.ndarray:
    nc = _get_compiled()
    in_maps = _shard_inputs(inputs)
    res = bass_utils.run_bass_kernel_spmd(
        nc, in_maps, core_ids=list(range(NCORES)), trace=False
    )
    acc = np.zeros((NTOK, H), np.float64)
    for r in res.results:
        acc += r["y"]
    return acc.astype(np.float32).reshape(np.asarray(inputs["hidden_states"]).shape)


# ---------------------------------------------------------------------------
# Timing helper (not used by the grader; test.py uses it to report HW time).
# Re-implements run_bass_via_pjrt's multi-core wiring but keeps the jitted
# callable so repeated executions stay device-resident and pipeline.
# ---------------------------------------------------------------------------


def measure_exec_ns(inputs, nrep: int = 24, check_against=None):
    import time

    import jax
    import jax.numpy as jnp  # noqa: F401
    from jax.sharding import Mesh, NamedSharding, PartitionSpec

    try:
        from jax.experimental.shard_map import shard_map
    except ImportError:
        from jax import shard_map  # type: ignore

    from concourse import bass2jax  # noqa: F401
    from concourse.bass2jax import (
        _bass_exec_p,
        install_neuronx_cc_hook,
        partition_id_tensor,
    )

    nc = _get_compiled()
    in_maps = _shard_inputs(inputs)
    install_neuronx_cc_hook()

    partition_name = nc.partition_id_tensor.name if nc.partition_id_tensor else None
    in_names: list[str] = []
    out_names: list[str] = []
    out_avals = []
    zero_outs = []
    for alloc in nc.m.functions[0].allocations:
        if not isinstance(alloc, mybir.MemoryLocationSet):
            continue
        name = alloc.memorylocations[0].name
        if alloc.kind == "ExternalInput":
            if name != partition_name:
                in_names.append(name)
        elif alloc.kind == "ExternalOutput":
            shape = tuple(alloc.tensor_shape)
            dtype = mybir.dt.np(alloc.dtype)
            out_names.append(name)
            out_avals.append(jax.core.ShapedArray(shape, dtype))
            zero_outs.append(np.zeros(shape, dtype))
    n_params = len(in_names)
    in_names = in_names + out_names
    if partition_name is not None:
        in_names = in_names + [partition_name]

    def _body(*args):
        operands = list(args)
        if partition_name is not None:
            operands.append(partition_id_tensor())
        outs = _bass_exec_p.bind(
            *operands,
            out_avals=tuple(out_avals),
            in_names=tuple(in_names),
            out_names=tuple(out_names),
            lowering_input_output_aliases=(),
            sim_require_finite=True,
            sim_require_nnan=True,
            nc=nc,
        )
        return tuple(outs)

    devices = jax.devices()[:NCORES]
    mesh = Mesh(np.asarray(devices), ("core",))
    spec = PartitionSpec("core")
    n_all = n_params + len(out_names)

    def _chained(n):
        # n sequential executions with a data dependency between them so the
        # effectful custom calls can't be CSE'd or overlapped; the slope of
        # total time vs n isolates true per-execution device time from the
        # (large) axon per-dispatch overhead.
        def _body_n(*args):
            args = list(args)
            outs = _body(*args)
            for _ in range(n - 1):
                eps = outs[0][0:1, 0:1] * 0.0
                args[0] = args[0] + eps.astype(args[0].dtype)
                outs = _body(*args)
            return outs

        return jax.jit(
            shard_map(
                _body_n,
                mesh=mesh,
                in_specs=(spec,) * n_all,
                out_specs=(spec,) * len(out_names),
                check_rep=False,
            ),
            keep_unused=True,
        )

    sharded = jax.jit(
        shard_map(
            _body,
            mesh=mesh,
            in_specs=(spec,) * n_all,
            out_specs=(spec,) * len(out_names),
            check_rep=False,
        ),
        keep_unused=True,
    )
    concat_in = [
        np.concatenate([np.asarray(in_maps[c][nm]) for c in range(NCORES)], axis=0)
        for nm in in_names[:n_params]
    ]
    concat_zeros = [
        np.zeros((NCORES * z.shape[0], *z.shape[1:]), z.dtype) for z in zero_outs
    ]
    shd = NamedSharding(mesh, spec)
    args = [jax.device_put(a, shd) for a in concat_in + concat_zeros]
    outs = sharded(*args)
    jax.block_until_ready(outs)
    if check_against is not None:
        got = np.asarray(outs[0]).reshape(NCORES, NTOK, H).sum(axis=0)
        err = np.max(np.abs(got - check_against)) / (
            np.max(np.abs(check_against)) + 1e-30
        )
        print(f"timing-path output relerr vs kernel(): {err:.3e}")
    del _chained  # chained custom calls are rejected by neuronx_cc_hook
    # Repeated async dispatch, amortized. This is an UPPER bound: each
    # dispatch pays the axon tunnel/PJRT overhead (~1ms+), which dwarfs the
    # device execution itself.
    t0 = time.perf_counter()
    pend = [sharded(*args) for _ in range(nrep)]
    jax.block_until_ready(pend)
    t1 = time.perf_counter()
    return (t1 - t0) / nrep * 1e9
